# revision 5
# baseline (speedup 1.0000x reference)
"""Trainium2 Bass kernel for the Involution module (B=4, C=64, H=W=128, K=7, G=4).

v2 architecture, per core (8-way data parallel: core = (batch, h-half)):
  - layout: partition p = c + 64*hb (hb = 32-row half), free = padded 38x134 slab.
  - Phase 1: 1x1 conv as 4 block-diag matmuls per chunk producing z in an
    (hb,g,k)-interleaved partition layout; fused BN+SiLU on ScalarE -> bf16 w
    tiles (k-position-major rows).
  - Phase 2 per 512-px chunk: for each k, ONE one-hot expansion matmul
    (n=128 covers both hb) row-tiled via tile_position so 4 k-positions run
    concurrently on the PE; per-pixel kernel products routed across engines
    (ScalarE copy + DVE 2x mult / DVE direct-PSUM mult / Pool mult); adds
    routed across PE (identity matmul accumulate into PSUM), DVE, Pool.
"""

import os

os.environ.setdefault("JAX_PLATFORMS", "cpu")

import numpy as np
import ml_dtypes

import concourse.bacc as bacc
import concourse.tile as tile
import concourse.mybir as mybir
from concourse.bass_utils import run_bass_kernel_spmd

# Problem constants (hardcoded per harness contract).
B, C, H, W = 4, 64, 128, 128
K, G, GC = 7, 4, 16
KK = K * K
KO = KK * G  # 196
PAD = 3
BN_EPS = 1e-5

HB_ROWS = 32          # rows per half-of-half (per partition group)
SLAB_R = HB_ROWS + 6  # 38 padded rows per hb slab
SLAB_W = W + 6        # 134 padded cols
SLAB_F = SLAB_R * SLAB_W
RPC = 4               # output rows per pixel chunk (512 px)
NCHUNK = HB_ROWS // RPC  # 8 chunks
CHW = RPC * W         # 512 free elements per chunk

# k-position chunking into 4 "m-chunks" of conv output rows.
MC_STARTS = [0, 13, 25, 37]
MC_SIZES = [13, 12, 12, 12]

USE_BF16 = True  # compat flag for test.py (bf16 is always on in v2)

bf16 = mybir.dt.bfloat16
f32 = mybir.dt.float32


def _k_decomp(k):
    for mc in range(3, -1, -1):
        if k >= MC_STARTS[mc]:
            k13 = k - MC_STARTS[mc]
            return mc, k13 // 4, k13 % 4  # (mc, jj, q)
    raise AssertionError


def _row(jj, q, hb, g):
    return 32 * q + 8 * jj + 4 * hb + g


# ---- engine routing tables (tunables) ----
def _routes():
    ap_ks = [k for k in range(KK) if k % 5 == 2]                 # Pool mults (10)
    rest = [k for k in range(KK) if k not in ap_ks]
    dd_ks = rest[::5]                                            # DVE direct (8)
    ad_ks = [k for k in rest if k not in dd_ks]                  # Act copy + DVE (31)
    pl_add = [k for k in range(KK) if k % 7 == 3]                # Pool adds (7)
    rest2 = [k for k in range(KK) if k not in pl_add]
    dv_add = rest2[::4]                                          # DVE adds (13)
    pe_add = [k for k in rest2 if k not in dv_add]               # PE adds (29)
    rm = {}
    for k in ap_ks:
        rm[k] = "AP"
    for k in dd_ks:
        rm[k] = "DD"
    for k in ad_ks:
        rm[k] = "AD"
    ra = {}
    for k in pl_add:
        ra[k] = "PL"
    for k in dv_add:
        ra[k] = "DV"
    for k in pe_add:
        ra[k] = "PE"
    return rm, ra


ROUTE_MULT, ROUTE_ADD = _routes()


def build_bass():
    nc = bacc.Bacc(
        "TRN2",
        target_bir_lowering=False,
        debug=False,
        enable_asserts=False,
        num_devices=8,
    )
    xq_d = nc.dram_tensor("xq", [128, SLAB_F], bf16, kind="ExternalInput").ap()
    S_d = nc.dram_tensor("S", [128, 4 * 128], bf16, kind="ExternalInput").ap()
    b_d = nc.dram_tensor("bv", [128, 4], f32, kind="ExternalInput").ap()
    E_d = nc.dram_tensor("E", [128, 4 * 128], bf16, kind="ExternalInput").ap()
    I_d = nc.dram_tensor("I", [128, 128], bf16, kind="ExternalInput").ap()
    out_d = nc.dram_tensor("out", [128, HB_ROWS * W], f32, kind="ExternalOutput").ap()

    with tile.TileContext(nc) as tc:
        build_kernel(tc, xq_d, S_d, b_d, E_d, I_d, out_d)
    nc.compile()
    return nc


def build_kernel(tc, xq_d, S_d, b_d, E_d, I_d, out_d):
    from contextlib import ExitStack

    nc = tc.nc
    silu = mybir.ActivationFunctionType.Silu

    ctx = ExitStack()
    consts = ctx.enter_context(tc.tile_pool(name="consts", bufs=1))
    wpool = ctx.enter_context(tc.tile_pool(name="w", bufs=1))
    wxpool = ctx.enter_context(tc.tile_pool(name="wx", bufs=4))
    prodpool = ctx.enter_context(tc.tile_pool(name="prod", bufs=6))
    accDpool = ctx.enter_context(tc.tile_pool(name="accD", bufs=2))
    accPpool = ctx.enter_context(tc.tile_pool(name="accP", bufs=2))
    outpool = ctx.enter_context(tc.tile_pool(name="outf", bufs=2))
    zpool = ctx.enter_context(tc.tile_pool(name="z", bufs=1, space="PSUM"))
    wepool = ctx.enter_context(tc.tile_pool(name="wex", bufs=1, space="PSUM"))
    accpool = ctx.enter_context(tc.tile_pool(name="acc", bufs=2, space="PSUM"))

    xq = consts.tile([128, SLAB_F], bf16)
    nc.sync.dma_start(out=xq, in_=xq_d)
    S = consts.tile([128, 4, 128], bf16)
    nc.sync.dma_start(out=S, in_=S_d.rearrange("p (m n) -> p m n", m=4))
    bvec = consts.tile([128, 4], f32)
    nc.sync.dma_start(out=bvec, in_=b_d)
    E = consts.tile([128, 4, 128], bf16)
    nc.sync.dma_start(out=E, in_=E_d.rearrange("p (m n) -> p m n", m=4))
    I128 = consts.tile([128, 128], bf16)
    nc.sync.dma_start(out=I128, in_=I_d)

    # element-shifted copy so odd-dw windows stay 4B-aligned (DVE 2x mode)
    xqo = consts.tile([128, SLAB_F], bf16)
    nc.vector.tensor_copy(xqo[:, 0 : SLAB_F - 2], xq[:, 1 : SLAB_F - 1])

    xq3 = xq.rearrange("p (r w) -> p r w", w=SLAB_W)
    xqo3 = xqo.rearrange("p (r w) -> p r w", w=SLAB_W)

    # ---- phase 1: conv + BN + SiLU -> w tiles (bf16, k-position-major) ----
    wt = {}
    for mc in range(4):
        for pair in range(4):
            z = zpool.tile([128, 2 * CHW], f32, tag="z")
            for hf in range(2):
                j = 2 * pair + hf
                rhs = xq3[:, RPC * j + PAD : RPC * j + PAD + RPC, PAD : PAD + W]
                nc.tensor.matmul(
                    z[:, hf * CHW : (hf + 1) * CHW],
                    S[:, mc, :],
                    rhs,
                    start=True,
                    stop=True,
                )
            w = wpool.tile([128, 2 * CHW], bf16, tag=f"w{mc}_{pair}")
            nc.scalar.activation(w, z, silu, bias=bvec[:, mc : mc + 1])
            wt[(mc, pair)] = w

    # ---- phase 2: per chunk, expansion + routed MAC ----
    for j in range(NCHUNK):
        pair, hf = j // 2, j % 2
        pe_ks = [k for k in range(KK) if ROUTE_ADD[k] == "PE"]
        acc = None
        if pe_ks:
            acc = accpool.tile([128, CHW], f32, tag="acc", name="acc")
        accD = None
        accP = None
        outf = outpool.tile([128, CHW], f32, tag="outf")

        pe_done = 0
        for mc in range(4):
            nquads = (MC_SIZES[mc] + 3) // 4
            for jj in range(nquads):
                ks = [
                    MC_STARTS[mc] + 4 * jj + q
                    for q in range(4)
                    if 4 * jj + q < MC_SIZES[mc]
                ]
                wexps = {}
                for k in ks:
                    _, _, q = _k_decomp(k)
                    wexp = wepool.tile([128, CHW], f32, tag=f"wex{q}")
                    nc.tensor.matmul(
                        wexp,
                        E[32 * q : 32 * q + 32, jj, :],
                        wt[(mc, pair)][32 * q : 32 * q + 32, hf * CHW : (hf + 1) * CHW],
                        start=True,
                        stop=True,
                        tile_position=(32 * q, 0),
                    )
                    wexps[k] = wexp

                for k in ks:
                    dh, dw = k // K, k % K
                    r0 = RPC * j + dh
                    if dw % 2 == 1:
                        xwin = xqo3[:, r0 : r0 + RPC, dw - 1 : dw - 1 + W]
                    else:
                        xwin = xq3[:, r0 : r0 + RPC, dw : dw + W]

                    rm = ROUTE_MULT[k]
                    ra = ROUTE_ADD[k]
                    if rm == "DD":
                        src = wexps[k]
                        meng = nc.vector
                    else:
                        wx = wxpool.tile([128, CHW], bf16, tag="wx")
                        nc.scalar.copy(wx, wexps[k])
                        src = wx
                        meng = nc.gpsimd if rm == "AP" else nc.vector

                    if ra == "DV" and accD is None:
                        accD = accDpool.tile([128, CHW], bf16, tag="accD")
                        meng.tensor_mul(accD, xwin, src)
                        continue
                    if ra == "PL" and accP is None:
                        accP = accPpool.tile([128, CHW], bf16, tag="accP")
                        meng.tensor_mul(accP, xwin, src)
                        continue

                    prod = prodpool.tile([128, CHW], bf16, tag="prod")
                    meng.tensor_mul(prod, xwin, src)

                    if ra == "PE":
                        pe_done += 1
                        nc.tensor.matmul(
                            acc,
                            I128,
                            prod,
                            start=(pe_done == 1),
                            stop=(pe_done == len(pe_ks)),
                        )
                    elif ra == "DV":
                        nc.vector.tensor_add(accD, accD, prod)
                    else:
                        nc.gpsimd.tensor_add(accP, accP, prod)

        # combine partial accumulators -> f32 -> DMA
        if accP is not None:
            nc.vector.tensor_add(accD, accD, accP)
        if acc is not None:
            nc.vector.tensor_add(outf, acc, accD)
        else:
            nc.vector.tensor_copy(outf, accD)
        nc.sync.dma_start(out=out_d[:, j * CHW : (j + 1) * CHW], in_=outf)
    ctx.close()


def prep_inputs(x, conv_w, bn_gamma, bn_beta, bn_mean, bn_var):
    """Host-side prep: per-core padded slabs + shared weight tables."""
    scale = (bn_gamma / np.sqrt(bn_var + BN_EPS)).astype(np.float32)
    shift = (bn_beta - bn_mean * scale).astype(np.float32)

    # conv stationaries S[mc]: [pin=(hb_in,c), pout=(q,jj,hb,g)] with BN scale folded
    S = np.zeros((128, 4, 128), np.float32)
    bvec = np.zeros((128, 4), np.float32)
    for mc in range(4):
        for jj in range(4):
            for q in range(4):
                k13 = 4 * jj + q
                if k13 >= MC_SIZES[mc]:
                    continue
                k = MC_STARTS[mc] + k13
                for hb in range(2):
                    for g in range(G):
                        r = _row(jj, q, hb, g)
                        ko = g * KK + k
                        S[64 * hb : 64 * hb + 64, mc, r] = conv_w[ko] * scale[ko]
                        bvec[r, mc] = shift[ko]

    # expansion one-hots: E[32q + 8jj' + 4hb + g, jj, 64hb + 16g + c16] = [jj'==jj]
    E = np.zeros((128, 4, 128), ml_dtypes.bfloat16)
    for q in range(4):
        for jj in range(4):
            for hb in range(2):
                for g in range(G):
                    r = _row(jj, q, hb, g)
                    c0 = 64 * hb + 16 * g
                    E[r, jj, c0 : c0 + 16] = 1.0

    I128 = np.eye(128, dtype=ml_dtypes.bfloat16)

    xp = np.zeros((B, C, H + 2 * PAD, W + 2 * PAD), ml_dtypes.bfloat16)
    xp[:, :, PAD : PAD + H, PAD : PAD + W] = x.astype(ml_dtypes.bfloat16)

    in_maps = []
    for core in range(8):
        b, half = core // 2, core % 2
        h0 = 64 * half
        xq = np.zeros((128, SLAB_F), ml_dtypes.bfloat16)
        for hb in range(2):
            r0 = h0 + HB_ROWS * hb
            slab = xp[b, :, r0 : r0 + SLAB_R, :]
            xq[64 * hb : 64 * hb + 64] = slab.reshape(C, SLAB_F)
        in_maps.append(
            {
                "xq": xq,
                "S": S.reshape(128, 512).astype(ml_dtypes.bfloat16),
                "bv": bvec,
                "E": E.reshape(128, 512),
                "I": I128,
            }
        )
    return in_maps


def assemble_output(results):
    out = np.zeros((B, C, H, W), np.float32)
    for core in range(8):
        b, half = core // 2, core % 2
        h0 = 64 * half
        oc = results[core]["out"].reshape(128, HB_ROWS, W)
        for hb in range(2):
            out[b, :, h0 + HB_ROWS * hb : h0 + HB_ROWS * (hb + 1), :] = oc[
                64 * hb : 64 * hb + 64
            ]
    return out


def kernel(x, conv_w, bn_gamma, bn_beta, bn_mean, bn_var):
    x = np.asarray(x, np.float32)
    conv_w = np.asarray(conv_w, np.float32)
    in_maps = prep_inputs(
        x,
        conv_w,
        np.asarray(bn_gamma, np.float32),
        np.asarray(bn_beta, np.float32),
        np.asarray(bn_mean, np.float32),
        np.asarray(bn_var, np.float32),
    )
    nc = build_bass()
    res = run_bass_kernel_spmd(nc, in_maps, core_ids=list(range(8)))
    return assemble_output(res.results)


if __name__ == "__main__":
    rng = np.random.default_rng(0)
    ins = {
        "x": rng.standard_normal((B, C, H, W), np.float32),
        "conv_w": rng.standard_normal((KO, C), np.float32) / 8.0,
        "bn_gamma": rng.uniform(0.5, 1.5, KO).astype(np.float32),
        "bn_beta": rng.standard_normal(KO).astype(np.float32) * 0.1,
        "bn_mean": rng.standard_normal(KO).astype(np.float32) * 0.1,
        "bn_var": rng.uniform(0.5, 1.5, KO).astype(np.float32),
    }
    out = kernel(**ins)
    print("kernel output", out.shape, out.dtype, np.abs(out).sum())
